# revision 8
# baseline (speedup 1.0000x reference)
"""Trainium2 Bass kernel for nn_AnmlLoss: contrastive-style loss over sim = feats @ feats.T.

Strategy (8 NeuronCores, data-parallel over rows of feats):
  - Host: build augmented transposed operands so one GEMM per core computes
        Mt = sim - G*eq          (G = 4.0)
    directly in PSUM:  lhs = [feats_shard.T ; -G*onehot_shard.T ; 0pad],
                       rhs = [feats.T       ;    onehot.T       ; 0pad].
    Same-class (eq) entries are pushed below -3, i.e. below every possible
    negative similarity, so per row:
        max_neg  = rowmax(Mt)
        neg_sum  = sum exp(40*Mt)          (eq terms underflow to 0)
        pexp     = exp(-2*Mt)              (eq terms carry an exact e^{2G} factor)
        pos mask = pexp > exp(-2*(thresh - G)),  thresh = min(1-eps, max_neg+margin)
        pos_sum  = e^{-2G} * sum(mask * pexp),   n_pos = sum(mask)
        n_neg    = B - class_count[label]  (via a tiny one-hot count GEMM)
  - Each core reduces its 512 rows to a single partial scalar; host sums the
    8 partials and divides by B.
"""

import numpy as np
import ml_dtypes
from contextlib import ExitStack

import concourse.tile as tile
from concourse import bacc, mybir
from concourse.bass_utils import run_bass_kernel_spmd

# problem constants (hardcoded per harness contract)
B, D, C = 4096, 1024, 64
NCORES = 8
R = B // NCORES            # 512 rows per core
P = 128                    # partitions
RT = R // P                # 4 row-tiles per core
NBW = 512                  # free-dim block width (one PSUM bank)
NB = B // NBW              # 8 col blocks
KAUG = 1152                # 1024 feats + 64 onehot + 64 zero pad
KC = KAUG // P             # 9 contraction chunks
OH_KC = 8                  # chunk index holding the one-hot rows (partitions 0..63)

G = 4.0
MARGIN = 0.09
EPS = 1e-5
C1 = float(np.exp(-2.0 * 0.501))
C2 = float(np.exp(40.0 * 0.531))
E_NEG2G = float(np.exp(-2.0 * G))

BF = mybir.dt.bfloat16
F32 = mybir.dt.float32


def _body(ctx, tc, out_d, rhs_d, lhs_d):
    nc = tc.nc
    AF = mybir.ActivationFunctionType
    ALU = mybir.AluOpType
    AX = mybir.AxisListType

    rhs_pool = ctx.enter_context(tc.tile_pool(name="rhs", bufs=KC * NB))
    lhs_pool = ctx.enter_context(tc.tile_pool(name="lhs", bufs=KC))
    pexp_pool = ctx.enter_context(tc.tile_pool(name="pexp", bufs=2))
    scr_pool = ctx.enter_context(tc.tile_pool(name="scr", bufs=3))
    parts_pool = ctx.enter_context(tc.tile_pool(name="parts", bufs=1))
    small_pool = ctx.enter_context(tc.tile_pool(name="small", bufs=1))
    rowst_pool = ctx.enter_context(tc.tile_pool(name="rowst", bufs=3))
    mt_pool = ctx.enter_context(tc.tile_pool(name="mt", bufs=4, space="PSUM"))
    psq_pool = ctx.enter_context(tc.tile_pool(name="psq", bufs=2, space="PSUM"))

    # ---- persistent inputs -------------------------------------------------
    lhs_sb = []
    for kc in range(KC):
        t = lhs_pool.tile([P, R], BF, tag=f"lhs{kc}")
        nc.sync.dma_start(out=t[:], in_=lhs_d[kc * P:(kc + 1) * P, :])
        lhs_sb.append(t)

    # rhs as separate [P, NBW] subtiles so deps stay per-block
    rhs_sb = [[None] * NB for _ in range(KC)]
    for nb in range(NB):
        for kc in range(KC):
            t = rhs_pool.tile([P, NBW], BF, tag="rhs")
            nc.sync.dma_start(
                out=t[:],
                in_=rhs_d[kc * P:(kc + 1) * P, nb * NBW:(nb + 1) * NBW],
            )
            rhs_sb[kc][nb] = t

    ones128 = small_pool.tile([P, 1], F32, tag="ones128")
    nc.vector.memset(ones128[:], 1.0)
    bias2g = small_pool.tile([P, 1], F32, tag="bias2g")
    nc.vector.memset(bias2g[:], 2.0 * G)
    biasc2 = small_pool.tile([P, 1], F32, tag="biasc2")
    nc.vector.memset(biasc2[:], C2)
    bias533 = small_pool.tile([P, 1], F32, tag="bias533")
    nc.vector.memset(bias533[:], 5.33)

    # per-(rowtile, block) partial stats, fp32
    mx_parts = parts_pool.tile([P, RT, NB], F32, tag="mx_parts")
    ns_parts = parts_pool.tile([P, RT, NB], F32, tag="ns_parts")
    ps_parts = parts_pool.tile([P, RT, NB], F32, tag="ps_parts")
    np_parts = parts_pool.tile([P, RT, NB], F32, tag="np_parts")
    cnt_parts = parts_pool.tile([64, NB], F32, tag="cnt_parts")

    # ---- main loop ---------------------------------------------------------
    for rt in range(RT):
        rsl = slice(rt * P, (rt + 1) * P)
        pexp_rt = pexp_pool.tile([P, B], BF, tag="pexp")

        # phase 1: GEMM + immediate per-block work
        for nb in range(NB):
            mt = mt_pool.tile([P, NBW], F32, tag="mt")
            for kc in range(KC):
                nc.tensor.matmul(
                    mt[:],
                    lhsT=lhs_sb[kc][:, rsl],
                    rhs=rhs_sb[kc][nb][:],
                    start=(kc == 0),
                    stop=(kc == KC - 1),
                )
            nsl = slice(nb * NBW, (nb + 1) * NBW)
            # pexp = exp(-2*Mt)  (bf16 store)
            nc.scalar.activation(out=pexp_rt[:, nsl], in_=mt[:], func=AF.Exp, scale=-2.0)
            # neg partial: sum exp(40*Mt)
            nscr = scr_pool.tile([P, NBW], BF, tag="nscr")
            nc.scalar.activation(
                out=nscr[:], in_=mt[:], func=AF.Exp, scale=40.0,
                accum_out=ns_parts[:, rt, nb:nb + 1],
            )
            # running row-max partial
            nc.vector.reduce_max(out=mx_parts[:, rt, nb:nb + 1], in_=mt[:], axis=AX.X)

        if rt == 0:
            # class counts: sum the one-hot rows (partitions 0..63 of chunk OH_KC)
            for nb in range(NB):
                cdead = scr_pool.tile([64, NBW], BF, tag="cdead")
                nc.scalar.activation(
                    out=cdead[:], in_=rhs_sb[OH_KC][nb][:64, :], func=AF.Copy,
                    accum_out=cnt_parts[:, nb:nb + 1],
                )

        # phase 2: row threshold then masked positive sums
        mx1 = rowst_pool.tile([P, 1], F32, tag="mx1")
        nc.vector.reduce_max(out=mx1[:], in_=mx_parts[:, rt, :], axis=AX.X)
        th = rowst_pool.tile([P, 1], F32, tag="th")
        nc.vector.tensor_scalar(
            out=th[:], in0=mx1[:], scalar1=MARGIN, scalar2=1.0 - EPS,
            op0=ALU.add, op1=ALU.min,
        )
        eth = rowst_pool.tile([P, 1], F32, tag="eth")
        nc.scalar.activation(out=eth[:], in_=th[:], func=AF.Exp, scale=-2.0, bias=bias2g[:])

        for nb in range(NB):
            nsl = slice(nb * NBW, (nb + 1) * NBW)
            pscr = scr_pool.tile([P, NBW], BF, tag="pscr")
            nc.vector.scalar_tensor_tensor(
                out=pscr[:], in0=pexp_rt[:, nsl], scalar=eth[:], in1=pexp_rt[:, nsl],
                op0=ALU.is_gt, op1=ALU.mult,
                accum_out=ps_parts[:, rt, nb:nb + 1],
            )
            cscr = scr_pool.tile([P, NBW], BF, tag="cscr")
            nc.vector.tensor_scalar(
                out=cscr[:], in0=pexp_rt[:, nsl], scalar1=eth[:], scalar2=None,
                op0=ALU.is_gt, op1=ALU.add,
                accum_out=np_parts[:, rt, nb:nb + 1],
            )

    # ---- epilogue (batched over the 4 row-tiles) ---------------------------
    cnt = small_pool.tile([64, 1], F32, tag="cnt")
    nc.vector.reduce_sum(out=cnt[:], in_=cnt_parts[:], axis=AX.X)
    cntb = small_pool.tile([P, 1], BF, tag="cntb")
    nc.vector.memset(cntb[:], 0.0)
    nc.scalar.copy(out=cntb[:64, :], in_=cnt[:])

    ns_r = small_pool.tile([P, RT], F32, tag="ns_r")
    ps_r = small_pool.tile([P, RT], F32, tag="ps_r")
    np_r = small_pool.tile([P, RT], F32, tag="np_r")
    eqs = small_pool.tile([P, RT], F32, tag="eqs")
    for rt in range(RT):
        nc.vector.reduce_sum(out=ns_r[:, rt:rt + 1], in_=ns_parts[:, rt, :], axis=AX.X)
        nc.vector.reduce_sum(out=ps_r[:, rt:rt + 1], in_=ps_parts[:, rt, :], axis=AX.X)
        nc.vector.reduce_sum(out=np_r[:, rt:rt + 1], in_=np_parts[:, rt, :], axis=AX.X)
        eqp = psq_pool.tile([P, 1], F32, tag="eqp")
        nc.tensor.matmul(
            eqp[:], lhsT=lhs_sb[OH_KC][:, rt * P:(rt + 1) * P], rhs=cntb[:],
            start=True, stop=True,
        )  # = -G * eqsum
        nc.scalar.copy(out=eqs[:, rt:rt + 1], in_=eqp[:])

    nneg = small_pool.tile([P, RT], F32, tag="nneg")
    nc.vector.tensor_scalar(
        out=nneg[:], in0=eqs[:], scalar1=1.0 / G, scalar2=float(B),
        op0=ALU.mult, op1=ALU.add,
    )  # B - eqsum
    pos_arg = small_pool.tile([P, RT], F32, tag="pos_arg")
    nc.vector.tensor_scalar(
        out=pos_arg[:], in0=ps_r[:], scalar1=E_NEG2G, scalar2=C1,
        op0=ALU.mult, op1=ALU.add,
    )

    lp1 = small_pool.tile([P, RT], F32, tag="lp1")
    nc.scalar.activation(out=lp1[:], in_=pos_arg[:], func=AF.Ln)
    lp2 = small_pool.tile([P, RT], F32, tag="lp2")
    nc.scalar.activation(out=lp2[:], in_=np_r[:], func=AF.Ln, bias=1.0)
    ln1 = small_pool.tile([P, RT], F32, tag="ln1")
    nc.scalar.activation(out=ln1[:], in_=ns_r[:], func=AF.Ln, bias=biasc2[:])
    ln2 = small_pool.tile([P, RT], F32, tag="ln2")
    nc.scalar.activation(out=ln2[:], in_=nneg[:], func=AF.Ln, bias=1.0)

    d1 = small_pool.tile([P, RT], F32, tag="d1")
    nc.vector.tensor_sub(d1[:], lp1[:], lp2[:])
    d2 = small_pool.tile([P, RT], F32, tag="d2")
    nc.vector.tensor_sub(d2[:], ln1[:], ln2[:])
    d2s = small_pool.tile([P, RT], F32, tag="d2s")
    nc.vector.tensor_scalar_mul(d2s[:], d2[:], 2.0 / 40.0)
    zh = small_pool.tile([P, RT], F32, tag="zh")
    nc.vector.tensor_add(zh[:], d1[:], d2s[:])  # 2*(pos_loss + neg_loss)
    z = small_pool.tile([P, RT], F32, tag="z")
    nc.vector.tensor_scalar_mul(z[:], zh[:], 0.5)
    ez = small_pool.tile([P, RT], F32, tag="ez")
    nc.scalar.activation(out=ez[:], in_=z[:], func=AF.Exp)
    pr = small_pool.tile([P, RT], F32, tag="pr")
    nc.scalar.activation(out=pr[:], in_=ez[:], func=AF.Ln, bias=bias533[:])

    m1 = small_pool.tile([P, RT], F32, tag="m1")
    nc.vector.scalar_tensor_tensor(
        out=m1[:], in0=np_r[:], scalar=0.5, in1=pr[:],
        op0=ALU.is_ge, op1=ALU.mult,
    )
    mkd = small_pool.tile([P, RT], F32, tag="mkd")
    nc.vector.scalar_tensor_tensor(
        out=mkd[:], in0=nneg[:], scalar=0.5, in1=m1[:],
        op0=ALU.is_ge, op1=ALU.mult,
    )

    lps = psq_pool.tile([1, RT], F32, tag="lps")
    nc.tensor.matmul(lps[:], lhsT=ones128[:], rhs=mkd[:], start=True, stop=True)
    outsb = small_pool.tile([1, 1], F32, tag="outsb")
    nc.vector.reduce_sum(out=outsb[:], in_=lps[:], axis=AX.X)
    nc.sync.dma_start(out=out_d[:, :], in_=outsb[:])


def build_graph():
    nc = bacc.Bacc("TRN2", target_bir_lowering=False, debug=False, num_devices=NCORES)
    rhs_d = nc.dram_tensor("rhs", [KAUG, B], BF, kind="ExternalInput").ap()
    lhs_d = nc.dram_tensor("lhs", [KAUG, R], BF, kind="ExternalInput").ap()
    out_d = nc.dram_tensor("out", [1, 1], F32, kind="ExternalOutput").ap()
    with tile.TileContext(nc) as tc:
        with ExitStack() as ctx:
            _body(ctx, tc, out_d, rhs_d, lhs_d)
    nc.compile()
    return nc


def prepare_in_maps(feats, labels):
    feats = np.ascontiguousarray(np.asarray(feats, dtype=np.float32))
    labels = np.asarray(labels).astype(np.int64)
    oh = np.zeros((B, C), np.float32)
    oh[np.arange(B), labels] = 1.0

    rhs = np.zeros((KAUG, B), np.float32)
    rhs[:D] = feats.T
    rhs[D:D + C] = oh.T
    rhs_bf = rhs.astype(ml_dtypes.bfloat16)

    in_maps = []
    for i in range(NCORES):
        sl = slice(i * R, (i + 1) * R)
        lhs = np.zeros((KAUG, R), np.float32)
        lhs[:D] = feats[sl].T
        lhs[D:D + C] = -G * oh[sl].T
        in_maps.append({"rhs": rhs_bf, "lhs": lhs.astype(ml_dtypes.bfloat16)})
    return in_maps


_cache = {}


def get_graph():
    if "nc" not in _cache:
        _cache["nc"] = build_graph()
    return _cache["nc"]


def kernel(**inputs):
    feats = inputs["feats"]
    labels = inputs["labels"]
    nc = get_graph()
    in_maps = prepare_in_maps(feats, labels)
    res = run_bass_kernel_spmd(nc, in_maps, core_ids=list(range(NCORES)))
    total = sum(float(r["out"][0, 0]) for r in res.results)
    return np.float32(total / B)


# revision 13
# speedup vs baseline: 1.1288x; 1.1288x over previous
"""Trainium2 Bass kernel for nn_AnmlLoss: contrastive-style loss over sim = feats @ feats.T.

Strategy (8 NeuronCores, data-parallel over rows of feats):
  - Host: build augmented transposed operands so one GEMM per core computes
        Mt = sim - G*eq          (G = 4.0)
    directly in PSUM:  lhs = [feats_shard.T ; -G*onehot_shard.T ; 0pad],
                       rhs = [feats.T       ;    onehot.T       ; 0pad].
    Same-class (eq) entries are pushed below -3, i.e. below every possible
    negative similarity, so per row:
        max_neg  = rowmax(Mt)
        neg_sum  = sum exp(40*Mt)          (eq terms underflow to 0)
        pexp     = exp(-2*Mt)              (eq terms carry an exact e^{2G} factor)
        pos mask = pexp > exp(-2*(thresh - G)),  thresh = min(1-eps, max_neg+margin)
        pos_sum_raw = sum(mask * pexp) = e^{2G} * pos_sum,   n_pos = sum(mask)
  - Device returns per-row (neg_sum, pos_sum_raw, n_pos); the host computes the
    per-row log epilogue (O(B) flops) and the final mean during unsharding.
  - GEMM loop is k-outer so the stationary operand is reused across 8 matmuls
    per LDWEIGHTS; post-GEMM ops run 1024 wide (2 PSUM banks) to amortize
    instruction overheads.
"""

import numpy as np
import ml_dtypes
from contextlib import ExitStack

import concourse.tile as tile
from concourse import bacc, mybir
from concourse.bass_utils import run_bass_kernel_spmd

# problem constants (hardcoded per harness contract)
B, D, C = 4096, 1024, 64
NCORES = 8
R = B // NCORES            # 512 rows per core
P = 128                    # partitions
RT = R // P                # 4 row-tiles per core
MMW = 512                  # matmul free width (one PSUM bank)
BW = 1024                  # post-GEMM block width (2 PSUM banks)
NB = B // BW               # 4 col blocks
KAUG = 1152                # 1024 feats + 64 onehot + 64 zero pad
KC = KAUG // P             # 9 contraction chunks

G = 4.0
MARGIN = 0.09
EPS = 1e-5
E_NEG2G = float(np.exp(-2.0 * G))

BF = mybir.dt.bfloat16
F32 = mybir.dt.float32


def _body(ctx, tc, out_d, rhs_d, lhs_d):
    nc = tc.nc
    AF = mybir.ActivationFunctionType
    ALU = mybir.AluOpType
    AX = mybir.AxisListType

    rhs_pool = ctx.enter_context(tc.tile_pool(name="rhs", bufs=KC * NB * 2))
    lhs_pool = ctx.enter_context(tc.tile_pool(name="lhs", bufs=KC))
    pexp_pool = ctx.enter_context(tc.tile_pool(name="pexp", bufs=2))
    scr_pool = ctx.enter_context(tc.tile_pool(name="scr", bufs=3))
    parts_pool = ctx.enter_context(tc.tile_pool(name="parts", bufs=1))
    small_pool = ctx.enter_context(tc.tile_pool(name="small", bufs=1))
    rowst_pool = ctx.enter_context(tc.tile_pool(name="rowst", bufs=3))
    mt_pool = ctx.enter_context(tc.tile_pool(name="mt", bufs=NB, space="PSUM"))

    # ---- persistent inputs -------------------------------------------------
    lhs_sb = []
    for kc in range(KC):
        t = lhs_pool.tile([P, R], BF, tag=f"lhs{kc}")
        nc.sync.dma_start(out=t[:], in_=lhs_d[kc * P:(kc + 1) * P, :])
        lhs_sb.append(t)

    # rhs as separate [P, MMW] subtiles so deps stay per-block
    rhs_sb = [[None] * (NB * 2) for _ in range(KC)]
    for mb in range(NB * 2):
        for kc in range(KC):
            t = rhs_pool.tile([P, MMW], BF, tag="rhs")
            nc.sync.dma_start(
                out=t[:],
                in_=rhs_d[kc * P:(kc + 1) * P, mb * MMW:(mb + 1) * MMW],
            )
            rhs_sb[kc][mb] = t

    bias2g = small_pool.tile([P, 1], F32, tag="bias2g")
    nc.vector.memset(bias2g[:], 2.0 * G)

    # per-(rowtile, block) partial stats, fp32
    mx_parts = parts_pool.tile([P, RT, NB], F32, tag="mx_parts")
    ns_parts = parts_pool.tile([P, RT, NB], F32, tag="ns_parts")
    ps_parts = parts_pool.tile([P, RT, NB], F32, tag="ps_parts")
    np_parts = parts_pool.tile([P, RT, NB], F32, tag="np_parts")

    out_sb = small_pool.tile([P, RT, 3], F32, tag="out_sb")

    # ---- main loop ---------------------------------------------------------
    for rt in range(RT):
        rsl = slice(rt * P, (rt + 1) * P)
        pexp_rt = pexp_pool.tile([P, B], BF, tag="pexp")

        # k-outer GEMM: one weight load per (rt, kc), 8 matmuls each
        mts = [mt_pool.tile([P, BW], F32, tag="mt", name=f"mt_{rt}_{i}") for i in range(NB)]
        for kc in range(KC):
            for mb in range(NB * 2):
                nc.tensor.matmul(
                    mts[mb // 2][:, (mb % 2) * MMW:(mb % 2 + 1) * MMW],
                    lhsT=lhs_sb[kc][:, rsl],
                    rhs=rhs_sb[kc][mb][:],
                    start=(kc == 0),
                    stop=(kc == KC - 1),
                )

        # phase 1 per 1024-wide block
        for nb in range(NB):
            mt = mts[nb]
            nsl = slice(nb * BW, (nb + 1) * BW)
            nc.scalar.activation(out=pexp_rt[:, nsl], in_=mt[:], func=AF.Exp, scale=-2.0)
            nscr = scr_pool.tile([P, BW], BF, tag="nscr")
            nc.scalar.activation(
                out=nscr[:], in_=mt[:], func=AF.Exp, scale=40.0,
                accum_out=ns_parts[:, rt, nb:nb + 1],
            )
            nc.vector.reduce_max(out=mx_parts[:, rt, nb:nb + 1], in_=mt[:], axis=AX.X)

        # phase 2: row threshold then masked positive sums
        mx1 = rowst_pool.tile([P, 1], F32, tag="mx1")
        nc.vector.reduce_max(out=mx1[:], in_=mx_parts[:, rt, :], axis=AX.X)
        th = rowst_pool.tile([P, 1], F32, tag="th")
        nc.vector.tensor_scalar(
            out=th[:], in0=mx1[:], scalar1=MARGIN, scalar2=1.0 - EPS,
            op0=ALU.add, op1=ALU.min,
        )
        eth = rowst_pool.tile([P, 1], F32, tag="eth")
        nc.scalar.activation(out=eth[:], in_=th[:], func=AF.Exp, scale=-2.0, bias=bias2g[:])

        for nb in range(NB):
            nsl = slice(nb * BW, (nb + 1) * BW)
            pscr = scr_pool.tile([P, BW], BF, tag="pscr")
            nc.vector.scalar_tensor_tensor(
                out=pscr[:], in0=pexp_rt[:, nsl], scalar=eth[:], in1=pexp_rt[:, nsl],
                op0=ALU.is_gt, op1=ALU.mult,
                accum_out=ps_parts[:, rt, nb:nb + 1],
            )
            cscr = scr_pool.tile([P, BW], BF, tag="cscr")
            nc.vector.tensor_scalar(
                out=cscr[:], in0=pexp_rt[:, nsl], scalar1=eth[:], scalar2=None,
                op0=ALU.is_gt, op1=ALU.add,
                accum_out=np_parts[:, rt, nb:nb + 1],
            )

    # ---- reduce partials into the output tile ------------------------------
    for rt in range(RT):
        nc.vector.reduce_sum(out=out_sb[:, rt, 0:1], in_=ns_parts[:, rt, :], axis=AX.X)
        nc.vector.reduce_sum(out=out_sb[:, rt, 1:2], in_=ps_parts[:, rt, :], axis=AX.X)
        nc.vector.reduce_sum(out=out_sb[:, rt, 2:3], in_=np_parts[:, rt, :], axis=AX.X)
    nc.sync.dma_start(out=out_d[:, :], in_=out_sb[:, :, :])


def build_graph():
    nc = bacc.Bacc("TRN2", target_bir_lowering=False, debug=False, num_devices=NCORES)
    rhs_d = nc.dram_tensor("rhs", [KAUG, B], BF, kind="ExternalInput").ap()
    lhs_d = nc.dram_tensor("lhs", [KAUG, R], BF, kind="ExternalInput").ap()
    out_d = nc.dram_tensor("out", [P, RT * 3], F32, kind="ExternalOutput").ap()
    with tile.TileContext(nc) as tc:
        with ExitStack() as ctx:
            _body(ctx, tc, out_d, rhs_d, lhs_d)
    nc.compile()
    return nc


def prepare_in_maps(feats, labels):
    feats = np.ascontiguousarray(np.asarray(feats, dtype=np.float32))
    labels = np.asarray(labels).astype(np.int64)
    oh = np.zeros((B, C), np.float32)
    oh[np.arange(B), labels] = 1.0

    rhs = np.zeros((KAUG, B), np.float32)
    rhs[:D] = feats.T
    rhs[D:D + C] = oh.T
    rhs_bf = rhs.astype(ml_dtypes.bfloat16)

    in_maps = []
    for i in range(NCORES):
        sl = slice(i * R, (i + 1) * R)
        lhs = np.zeros((KAUG, R), np.float32)
        lhs[:D] = feats[sl].T
        lhs[D:D + C] = -G * oh[sl].T
        in_maps.append({"rhs": rhs_bf, "lhs": lhs.astype(ml_dtypes.bfloat16)})
    return in_maps


def host_epilogue(outs, labels):
    """Per-row log epilogue + mean, from per-row (neg_sum, pos_sum_raw, n_pos)."""
    labels = np.asarray(labels).astype(np.int64)
    counts = np.bincount(labels, minlength=C)
    n_neg = (B - counts[labels]).astype(np.float64)        # [B]

    # outs[i]: [P, RT*3] for rows i*R + rt*P + p
    ns = np.empty(B); ps_raw = np.empty(B); npos = np.empty(B)
    for i, o in enumerate(outs):
        o = np.asarray(o, np.float64).reshape(P, RT, 3)
        for rt in range(RT):
            rows = slice(i * R + rt * P, i * R + (rt + 1) * P)
            ns[rows] = o[:, rt, 0]
            ps_raw[rows] = o[:, rt, 1]
            npos[rows] = o[:, rt, 2]

    pos_sum = ps_raw * E_NEG2G
    pos_loss = 0.5 * np.log((pos_sum + np.exp(-2.0 * 0.501)) / (npos + 1.0))
    neg_loss = (1.0 / 40.0) * np.log((ns + np.exp(40.0 * 0.531)) / (n_neg + 1.0))
    per_row = np.log(5.33 + np.exp(pos_loss + neg_loss))
    valid = (npos >= 0.5) & (n_neg >= 0.5)
    return float(np.where(valid, per_row, 0.0).sum() / B)


_cache = {}


def get_graph():
    if "nc" not in _cache:
        _cache["nc"] = build_graph()
    return _cache["nc"]


def kernel(**inputs):
    feats = inputs["feats"]
    labels = inputs["labels"]
    nc = get_graph()
    in_maps = prepare_in_maps(feats, labels)
    res = run_bass_kernel_spmd(nc, in_maps, core_ids=list(range(NCORES)))
    return np.float32(host_epilogue([r["out"] for r in res.results], labels))


# revision 14
# speedup vs baseline: 1.2760x; 1.1304x over previous
"""Trainium2 Bass kernel for nn_AnmlLoss: contrastive-style loss over sim = feats @ feats.T.

Strategy (8 NeuronCores, data-parallel over rows of feats):
  - Host: build augmented transposed operands so one GEMM per core computes
        Mt = sim - G*eq          (G = 4.0)
    directly in PSUM:  lhs = [feats_shard.T ; -G*onehot_shard.T ; 0pad],
                       rhs = [feats.T       ;    onehot.T       ; 0pad].
    Same-class (eq) entries are pushed below -3, i.e. below every possible
    negative similarity, so per row:
        max_neg  = rowmax(Mt)
        neg_sum  = sum exp(40*Mt)          (eq terms underflow to 0)
        pexp     = exp(-2*Mt)              (eq terms carry an exact e^{2G} factor)
        pos mask = pexp > exp(-2*(thresh - G)),  thresh = min(1-eps, max_neg+margin)
        pos_sum_raw = sum(mask * pexp) = e^{2G} * pos_sum,   n_pos = sum(mask)
  - Device returns per-row (neg_sum, pos_sum_raw, n_pos); the host computes the
    per-row log epilogue (O(B) flops) and the final mean during unsharding.
  - GEMM loop is k-outer so the stationary operand is reused across 8 matmuls
    per LDWEIGHTS; post-GEMM ops run 1024 wide (2 PSUM banks) to amortize
    instruction overheads.
"""

import numpy as np
import ml_dtypes
from contextlib import ExitStack

import concourse.tile as tile
from concourse import bacc, mybir
from concourse.bass_utils import run_bass_kernel_spmd

# problem constants (hardcoded per harness contract)
B, D, C = 4096, 1024, 64
NCORES = 8
R = B // NCORES            # 512 rows per core
P = 128                    # partitions
RT = R // P                # 4 row-tiles per core
MMW = 512                  # matmul free width (one PSUM bank)
BW = 1024                  # post-GEMM block width (2 PSUM banks)
NB = B // BW               # 4 col blocks
KAUG = 1152                # 1024 feats + 64 onehot + 64 zero pad
KC = KAUG // P             # 9 contraction chunks

G = 4.0
MARGIN = 0.09
EPS = 1e-5
E_NEG2G = float(np.exp(-2.0 * G))

BF = mybir.dt.bfloat16
F32 = mybir.dt.float32


def _body(ctx, tc, out_d, rhs_d, lhs_d):
    nc = tc.nc
    AF = mybir.ActivationFunctionType
    ALU = mybir.AluOpType
    AX = mybir.AxisListType

    rhs_pool = ctx.enter_context(tc.tile_pool(name="rhs", bufs=KC * NB * 2))
    lhs_pool = ctx.enter_context(tc.tile_pool(name="lhs", bufs=KC))
    pexp_pool = ctx.enter_context(tc.tile_pool(name="pexp", bufs=2))
    scr_pool = ctx.enter_context(tc.tile_pool(name="scr", bufs=3))
    parts_pool = ctx.enter_context(tc.tile_pool(name="parts", bufs=1))
    small_pool = ctx.enter_context(tc.tile_pool(name="small", bufs=1))
    rowst_pool = ctx.enter_context(tc.tile_pool(name="rowst", bufs=3))
    mt_pool = ctx.enter_context(tc.tile_pool(name="mt", bufs=NB, space="PSUM"))

    # ---- persistent inputs -------------------------------------------------
    lhs_sb = []
    for kc in range(KC):
        t = lhs_pool.tile([P, R], BF, tag=f"lhs{kc}")
        nc.sync.dma_start(out=t[:], in_=lhs_d[kc * P:(kc + 1) * P, :])
        lhs_sb.append(t)

    # rhs as separate [P, MMW] subtiles so deps stay per-block
    rhs_sb = [[None] * (NB * 2) for _ in range(KC)]
    for mb in range(NB * 2):
        for kc in range(KC):
            t = rhs_pool.tile([P, MMW], BF, tag="rhs")
            nc.sync.dma_start(
                out=t[:],
                in_=rhs_d[kc * P:(kc + 1) * P, mb * MMW:(mb + 1) * MMW],
            )
            rhs_sb[kc][mb] = t

    bias2g = small_pool.tile([P, 1], F32, tag="bias2g")
    nc.vector.memset(bias2g[:], 2.0 * G)

    # per-(rowtile, block) partial stats, fp32
    mx_parts = parts_pool.tile([P, RT, NB], F32, tag="mx_parts")
    ns_parts = parts_pool.tile([P, RT, NB], F32, tag="ns_parts")
    ps_parts = parts_pool.tile([P, RT, NB], F32, tag="ps_parts")
    np_parts = parts_pool.tile([P, RT, NB], F32, tag="np_parts")

    out_sb = small_pool.tile([P, RT, 3], F32, tag="out_sb")

    # ---- main loop ---------------------------------------------------------
    for rt in range(RT):
        rsl = slice(rt * P, (rt + 1) * P)
        pexp_rt = pexp_pool.tile([P, B], BF, tag="pexp")

        # k-outer GEMM: one weight load per (rt, kc), 8 matmuls each
        mts = [mt_pool.tile([P, BW], F32, tag="mt", name=f"mt_{rt}_{i}") for i in range(NB)]
        for mb in range(NB * 2):
            for kc in range(KC):
                nc.tensor.matmul(
                    mts[mb // 2][:, (mb % 2) * MMW:(mb % 2 + 1) * MMW],
                    lhsT=lhs_sb[kc][:, rsl],
                    rhs=rhs_sb[kc][mb][:],
                    start=(kc == 0),
                    stop=(kc == KC - 1),
                )

        # phase 1 per 1024-wide block
        for nb in range(NB):
            mt = mts[nb]
            nsl = slice(nb * BW, (nb + 1) * BW)
            nc.scalar.activation(out=pexp_rt[:, nsl], in_=mt[:], func=AF.Exp, scale=-2.0)
            nscr = scr_pool.tile([P, BW], BF, tag="nscr")
            nc.scalar.activation(
                out=nscr[:], in_=mt[:], func=AF.Exp, scale=40.0,
                accum_out=ns_parts[:, rt, nb:nb + 1],
            )
            nc.vector.reduce_max(out=mx_parts[:, rt, nb:nb + 1], in_=mt[:], axis=AX.X)

        # phase 2: row threshold then masked positive sums
        mx1 = rowst_pool.tile([P, 1], F32, tag="mx1")
        nc.vector.reduce_max(out=mx1[:], in_=mx_parts[:, rt, :], axis=AX.X)
        th = rowst_pool.tile([P, 1], F32, tag="th")
        nc.vector.tensor_scalar(
            out=th[:], in0=mx1[:], scalar1=MARGIN, scalar2=1.0 - EPS,
            op0=ALU.add, op1=ALU.min,
        )
        eth = rowst_pool.tile([P, 1], F32, tag="eth")
        nc.scalar.activation(out=eth[:], in_=th[:], func=AF.Exp, scale=-2.0, bias=bias2g[:])

        for nb in range(NB):
            nsl = slice(nb * BW, (nb + 1) * BW)
            pscr = scr_pool.tile([P, BW], BF, tag="pscr")
            nc.vector.scalar_tensor_tensor(
                out=pscr[:], in0=pexp_rt[:, nsl], scalar=eth[:], in1=pexp_rt[:, nsl],
                op0=ALU.is_gt, op1=ALU.mult,
                accum_out=ps_parts[:, rt, nb:nb + 1],
            )
            cscr = scr_pool.tile([P, BW], BF, tag="cscr")
            nc.vector.tensor_scalar(
                out=cscr[:], in0=pexp_rt[:, nsl], scalar1=eth[:], scalar2=None,
                op0=ALU.is_gt, op1=ALU.add,
                accum_out=np_parts[:, rt, nb:nb + 1],
            )

    # ---- reduce partials into the output tile ------------------------------
    for rt in range(RT):
        nc.vector.reduce_sum(out=out_sb[:, rt, 0:1], in_=ns_parts[:, rt, :], axis=AX.X)
        nc.vector.reduce_sum(out=out_sb[:, rt, 1:2], in_=ps_parts[:, rt, :], axis=AX.X)
        nc.vector.reduce_sum(out=out_sb[:, rt, 2:3], in_=np_parts[:, rt, :], axis=AX.X)
    nc.sync.dma_start(out=out_d[:, :], in_=out_sb[:, :, :])


def build_graph():
    nc = bacc.Bacc("TRN2", target_bir_lowering=False, debug=False, num_devices=NCORES)
    rhs_d = nc.dram_tensor("rhs", [KAUG, B], BF, kind="ExternalInput").ap()
    lhs_d = nc.dram_tensor("lhs", [KAUG, R], BF, kind="ExternalInput").ap()
    out_d = nc.dram_tensor("out", [P, RT * 3], F32, kind="ExternalOutput").ap()
    with tile.TileContext(nc) as tc:
        with ExitStack() as ctx:
            _body(ctx, tc, out_d, rhs_d, lhs_d)
    nc.compile()
    return nc


def prepare_in_maps(feats, labels):
    feats = np.ascontiguousarray(np.asarray(feats, dtype=np.float32))
    labels = np.asarray(labels).astype(np.int64)
    oh = np.zeros((B, C), np.float32)
    oh[np.arange(B), labels] = 1.0

    rhs = np.zeros((KAUG, B), np.float32)
    rhs[:D] = feats.T
    rhs[D:D + C] = oh.T
    rhs_bf = rhs.astype(ml_dtypes.bfloat16)

    in_maps = []
    for i in range(NCORES):
        sl = slice(i * R, (i + 1) * R)
        lhs = np.zeros((KAUG, R), np.float32)
        lhs[:D] = feats[sl].T
        lhs[D:D + C] = -G * oh[sl].T
        in_maps.append({"rhs": rhs_bf, "lhs": lhs.astype(ml_dtypes.bfloat16)})
    return in_maps


def host_epilogue(outs, labels):
    """Per-row log epilogue + mean, from per-row (neg_sum, pos_sum_raw, n_pos)."""
    labels = np.asarray(labels).astype(np.int64)
    counts = np.bincount(labels, minlength=C)
    n_neg = (B - counts[labels]).astype(np.float64)        # [B]

    # outs[i]: [P, RT*3] for rows i*R + rt*P + p
    ns = np.empty(B); ps_raw = np.empty(B); npos = np.empty(B)
    for i, o in enumerate(outs):
        o = np.asarray(o, np.float64).reshape(P, RT, 3)
        for rt in range(RT):
            rows = slice(i * R + rt * P, i * R + (rt + 1) * P)
            ns[rows] = o[:, rt, 0]
            ps_raw[rows] = o[:, rt, 1]
            npos[rows] = o[:, rt, 2]

    pos_sum = ps_raw * E_NEG2G
    pos_loss = 0.5 * np.log((pos_sum + np.exp(-2.0 * 0.501)) / (npos + 1.0))
    neg_loss = (1.0 / 40.0) * np.log((ns + np.exp(40.0 * 0.531)) / (n_neg + 1.0))
    per_row = np.log(5.33 + np.exp(pos_loss + neg_loss))
    valid = (npos >= 0.5) & (n_neg >= 0.5)
    return float(np.where(valid, per_row, 0.0).sum() / B)


_cache = {}


def get_graph():
    if "nc" not in _cache:
        _cache["nc"] = build_graph()
    return _cache["nc"]


def kernel(**inputs):
    feats = inputs["feats"]
    labels = inputs["labels"]
    nc = get_graph()
    in_maps = prepare_in_maps(feats, labels)
    res = run_bass_kernel_spmd(nc, in_maps, core_ids=list(range(NCORES)))
    return np.float32(host_epilogue([r["out"] for r in res.results], labels))


# revision 17
# speedup vs baseline: 1.2981x; 1.0173x over previous
"""Trainium2 Bass kernel for nn_AnmlLoss: contrastive-style loss over sim = feats @ feats.T.

Strategy (8 NeuronCores, data-parallel over rows of feats):
  - Host: build augmented transposed operands so one GEMM per core computes
        Mt = sim - G*eq          (G = 4.0)
    directly in PSUM:  lhs = [feats_shard.T ; -G*onehot_shard.T ; 0pad],
                       rhs = [feats.T       ;    onehot.T       ; 0pad].
    Same-class (eq) entries are pushed below -3, i.e. below every possible
    negative similarity, so per row:
        max_neg  = rowmax(Mt)
        neg_sum  = sum exp(40*Mt)          (eq terms underflow to 0)
        pexp     = exp(-2*Mt)              (eq terms carry an exact e^{2G} factor)
        pos mask = pexp > exp(-2*(thresh - G)),  thresh = min(1-eps, max_neg+margin)
        pos_sum_raw = sum(mask * pexp) = e^{2G} * pos_sum,   n_pos = sum(mask)
  - Device returns per-row (neg_sum, pos_sum_raw, n_pos); the host computes the
    per-row log epilogue (O(B) flops) and the final mean during unsharding.
  - GEMM loop is k-outer so the stationary operand is reused across 8 matmuls
    per LDWEIGHTS; post-GEMM ops run 1024 wide (2 PSUM banks) to amortize
    instruction overheads.
"""

import numpy as np
import ml_dtypes
from contextlib import ExitStack

import concourse.tile as tile
from concourse import bacc, mybir
from concourse.bass_utils import run_bass_kernel_spmd

# problem constants (hardcoded per harness contract)
B, D, C = 4096, 1024, 64
NCORES = 8
R = B // NCORES            # 512 rows per core
P = 128                    # partitions
RT = R // P                # 4 row-tiles per core
MMW = 512                  # matmul free width (one PSUM bank)
BW = 1024                  # post-GEMM block width (2 PSUM banks)
NB = B // BW               # 4 col blocks
KAUG = 1152                # 1024 feats + 64 onehot + 64 zero pad
KC = KAUG // P             # 9 contraction chunks

G = 4.0
MARGIN = 0.09
EPS = 1e-5
E_NEG2G = float(np.exp(-2.0 * G))

BF = mybir.dt.bfloat16
F32 = mybir.dt.float32


def _body(ctx, tc, out_d, rhs_d, lhs_d):
    nc = tc.nc
    AF = mybir.ActivationFunctionType
    ALU = mybir.AluOpType
    AX = mybir.AxisListType

    rhs_pool = ctx.enter_context(tc.tile_pool(name="rhs", bufs=KC * NB * 2))
    lhs_pool = ctx.enter_context(tc.tile_pool(name="lhs", bufs=KC))
    pexp_pool = ctx.enter_context(tc.tile_pool(name="pexp", bufs=2))
    scr_pool = ctx.enter_context(tc.tile_pool(name="scr", bufs=3))
    parts_pool = ctx.enter_context(tc.tile_pool(name="parts", bufs=1))
    small_pool = ctx.enter_context(tc.tile_pool(name="small", bufs=1))
    rowst_pool = ctx.enter_context(tc.tile_pool(name="rowst", bufs=3))
    mt_pool = ctx.enter_context(tc.tile_pool(name="mt", bufs=NB, space="PSUM"))

    # ---- persistent inputs -------------------------------------------------
    # lhs chunks go on the gpsimd DMA queue so the sync queue drains the
    # first matmul's rhs blocks as early as possible
    lhs_sb = []
    for kc in range(KC):
        t = lhs_pool.tile([P, R], BF, tag=f"lhs{kc}")
        nc.gpsimd.dma_start(out=t[:], in_=lhs_d[kc * P:(kc + 1) * P, :])
        lhs_sb.append(t)

    # rhs as separate [P, MMW] subtiles so deps stay per-block; split across
    # the sync (even mb) and vector (odd mb) DMA queues
    rhs_sb = [[None] * (NB * 2) for _ in range(KC)]
    for mb in range(NB * 2):
        eng = nc.sync if mb % 2 == 0 else nc.scalar
        for kc in range(KC):
            t = rhs_pool.tile([P, MMW], BF, tag="rhs")
            eng.dma_start(
                out=t[:],
                in_=rhs_d[kc * P:(kc + 1) * P, mb * MMW:(mb + 1) * MMW],
            )
            rhs_sb[kc][mb] = t

    bias2g = small_pool.tile([P, 1], F32, tag="bias2g")
    nc.vector.memset(bias2g[:], 2.0 * G)

    # per-(rowtile, block) partial stats, fp32
    mx_parts = parts_pool.tile([P, RT, NB], F32, tag="mx_parts")
    ns_parts = parts_pool.tile([P, RT, NB], F32, tag="ns_parts")
    ps_parts = parts_pool.tile([P, RT, NB], F32, tag="ps_parts")
    np_parts = parts_pool.tile([P, RT, NB], F32, tag="np_parts")

    out_sb = small_pool.tile([P, RT, 3], F32, tag="out_sb")

    # ---- main loop ---------------------------------------------------------
    for rt in range(RT):
        rsl = slice(rt * P, (rt + 1) * P)
        pexp_rt = pexp_pool.tile([P, B], BF, tag="pexp")

        # k-outer GEMM: one weight load per (rt, kc), 8 matmuls each
        # one weight load serves both 512-halves of each 1024-wide block
        mts = [mt_pool.tile([P, BW], F32, tag="mt", name=f"mt_{rt}_{i}") for i in range(NB)]
        for nb in range(NB):
            for kc in range(KC):
                for h in range(2):
                    nc.tensor.matmul(
                        mts[nb][:, h * MMW:(h + 1) * MMW],
                        lhsT=lhs_sb[kc][:, rsl],
                        rhs=rhs_sb[kc][nb * 2 + h][:],
                        start=(kc == 0),
                        stop=(kc == KC - 1),
                    )

        # phase 1 per 1024-wide block
        for nb in range(NB):
            mt = mts[nb]
            nsl = slice(nb * BW, (nb + 1) * BW)
            nc.scalar.activation(out=pexp_rt[:, nsl], in_=mt[:], func=AF.Exp, scale=-2.0)
            nscr = scr_pool.tile([P, BW], BF, tag="nscr")
            nc.scalar.activation(
                out=nscr[:], in_=mt[:], func=AF.Exp, scale=40.0,
                accum_out=ns_parts[:, rt, nb:nb + 1],
            )
            nc.vector.reduce_max(out=mx_parts[:, rt, nb:nb + 1], in_=mt[:], axis=AX.X)

        # phase 2: row threshold then masked positive sums
        mx1 = rowst_pool.tile([P, 1], F32, tag="mx1")
        nc.vector.reduce_max(out=mx1[:], in_=mx_parts[:, rt, :], axis=AX.X)
        th = rowst_pool.tile([P, 1], F32, tag="th")
        nc.vector.tensor_scalar(
            out=th[:], in0=mx1[:], scalar1=MARGIN, scalar2=1.0 - EPS,
            op0=ALU.add, op1=ALU.min,
        )
        eth = rowst_pool.tile([P, 1], F32, tag="eth")
        nc.scalar.activation(out=eth[:], in_=th[:], func=AF.Exp, scale=-2.0, bias=bias2g[:])

        for nb in range(NB):
            nsl = slice(nb * BW, (nb + 1) * BW)
            pscr = scr_pool.tile([P, BW], BF, tag="pscr")
            nc.vector.scalar_tensor_tensor(
                out=pscr[:], in0=pexp_rt[:, nsl], scalar=eth[:], in1=pexp_rt[:, nsl],
                op0=ALU.is_gt, op1=ALU.mult,
                accum_out=ps_parts[:, rt, nb:nb + 1],
            )
            cscr = scr_pool.tile([P, BW], BF, tag="cscr")
            nc.vector.tensor_scalar(
                out=cscr[:], in0=pexp_rt[:, nsl], scalar1=eth[:], scalar2=None,
                op0=ALU.is_gt, op1=ALU.add,
                accum_out=np_parts[:, rt, nb:nb + 1],
            )

    # ---- reduce partials into the output tile ------------------------------
    for rt in range(RT):
        nc.vector.reduce_sum(out=out_sb[:, rt, 0:1], in_=ns_parts[:, rt, :], axis=AX.X)
        nc.vector.reduce_sum(out=out_sb[:, rt, 1:2], in_=ps_parts[:, rt, :], axis=AX.X)
        nc.vector.reduce_sum(out=out_sb[:, rt, 2:3], in_=np_parts[:, rt, :], axis=AX.X)
    nc.sync.dma_start(out=out_d[:, :], in_=out_sb[:, :, :])


def build_graph():
    nc = bacc.Bacc("TRN2", target_bir_lowering=False, debug=False, num_devices=NCORES)
    rhs_d = nc.dram_tensor("rhs", [KAUG, B], BF, kind="ExternalInput").ap()
    lhs_d = nc.dram_tensor("lhs", [KAUG, R], BF, kind="ExternalInput").ap()
    out_d = nc.dram_tensor("out", [P, RT * 3], F32, kind="ExternalOutput").ap()
    with tile.TileContext(nc) as tc:
        with ExitStack() as ctx:
            _body(ctx, tc, out_d, rhs_d, lhs_d)
    nc.compile()
    return nc


def prepare_in_maps(feats, labels):
    feats = np.ascontiguousarray(np.asarray(feats, dtype=np.float32))
    labels = np.asarray(labels).astype(np.int64)
    oh = np.zeros((B, C), np.float32)
    oh[np.arange(B), labels] = 1.0

    rhs = np.zeros((KAUG, B), np.float32)
    rhs[:D] = feats.T
    rhs[D:D + C] = oh.T
    rhs_bf = rhs.astype(ml_dtypes.bfloat16)

    in_maps = []
    for i in range(NCORES):
        sl = slice(i * R, (i + 1) * R)
        lhs = np.zeros((KAUG, R), np.float32)
        lhs[:D] = feats[sl].T
        lhs[D:D + C] = -G * oh[sl].T
        in_maps.append({"rhs": rhs_bf, "lhs": lhs.astype(ml_dtypes.bfloat16)})
    return in_maps


def host_epilogue(outs, labels):
    """Per-row log epilogue + mean, from per-row (neg_sum, pos_sum_raw, n_pos)."""
    labels = np.asarray(labels).astype(np.int64)
    counts = np.bincount(labels, minlength=C)
    n_neg = (B - counts[labels]).astype(np.float64)        # [B]

    # outs[i]: [P, RT*3] for rows i*R + rt*P + p
    ns = np.empty(B); ps_raw = np.empty(B); npos = np.empty(B)
    for i, o in enumerate(outs):
        o = np.asarray(o, np.float64).reshape(P, RT, 3)
        for rt in range(RT):
            rows = slice(i * R + rt * P, i * R + (rt + 1) * P)
            ns[rows] = o[:, rt, 0]
            ps_raw[rows] = o[:, rt, 1]
            npos[rows] = o[:, rt, 2]

    pos_sum = ps_raw * E_NEG2G
    pos_loss = 0.5 * np.log((pos_sum + np.exp(-2.0 * 0.501)) / (npos + 1.0))
    neg_loss = (1.0 / 40.0) * np.log((ns + np.exp(40.0 * 0.531)) / (n_neg + 1.0))
    per_row = np.log(5.33 + np.exp(pos_loss + neg_loss))
    valid = (npos >= 0.5) & (n_neg >= 0.5)
    return float(np.where(valid, per_row, 0.0).sum() / B)


_cache = {}


def get_graph():
    if "nc" not in _cache:
        _cache["nc"] = build_graph()
    return _cache["nc"]


def kernel(**inputs):
    feats = inputs["feats"]
    labels = inputs["labels"]
    nc = get_graph()
    in_maps = prepare_in_maps(feats, labels)
    res = run_bass_kernel_spmd(nc, in_maps, core_ids=list(range(NCORES)))
    return np.float32(host_epilogue([r["out"] for r in res.results], labels))
